# revision 42
# baseline (speedup 1.0000x reference)
"""Trainium2 Bass kernel for nn_DecoderHead (MAE-style decoder head).

Strategy (8 NeuronCores): data-parallel over batch B=4 x 2-way token split
per batch. Cores sharing a batch both compute layer 0 K/V for all 2048
tokens, then exchange x1 halves (feature-major, via AllGather + indirect
readback) so layer 1 + head run on their own 1024-token half.

On-device layout is feature-major (x^T: [D, T] with D on partitions).
Input scatter and output transpose are done host-side; the device input is
x0^T pre-permuted per core. Heads padded 96->128; a ones-row injected in V
(via bias) makes PV emit softmax denominators for free. LN gamma/beta are
folded into adjacent weights host-side. All GEMMs run in bf16 (fp8e4
DoubleRow paths exist behind USE_FP8_* flags but e4m3 noise exceeds the
harness tolerance); PSUM is organized as 2-bank pair tiles (3+1 rotation)
so evictions are 1024 wide and the PE stays deep in its pipeline.
"""

import sys
import numpy as np

sys.path.insert(0, "/opt/trn_rl_repo")

import ml_dtypes

P = 128
B = 4
N_VIS = 512
T = 2048          # N_TOT
D = 768
KD = D // P       # 6
NH = 8
DH = 96
HID = 3072
HB = HID // P     # 24
DEPTH = 2
TQ = 1024         # own-half tokens per core
CH = 512          # token chunk
Q0 = T - TQ
EPS = 1e-5

BF16 = ml_dtypes.bfloat16
F8 = ml_dtypes.float8_e4m3

USE_FP8_QKV = False
USE_FP8_FFN = False

_cache = {}


def _build():
    import concourse.bass as bass
    import concourse.mybir as mybir
    import concourse.tile as tile
    from concourse import bacc
    from concourse.masks import make_identity

    dt = mybir.dt
    nc = bacc.Bacc("TRN2", target_bir_lowering=False, debug=False, num_devices=8)

    f8 = dt.float8e4
    qkv_wdt = f8 if USE_FP8_QKV else dt.bfloat16
    ffn_wdt = f8 if USE_FP8_FFN else dt.bfloat16

    x0 = nc.dram_tensor("x0", (KD, P, T), dt.float16, kind="ExternalInput").ap()
    pidx = nc.dram_tensor("pidx", (KD * P, 1), dt.int32, kind="ExternalInput").ap()
    wqkv = nc.dram_tensor("wqkv", (DEPTH, D, NH * 3 * P), qkv_wdt, kind="ExternalInput").ap()
    bqkv = nc.dram_tensor("bqkv", (DEPTH, NH * 3, P), dt.float32, kind="ExternalInput").ap()
    sqkv = nc.dram_tensor("sqkv", (DEPTH, NH * 3, P), dt.float32, kind="ExternalInput").ap()
    wo = nc.dram_tensor("wo", (DEPTH, NH * P, D), dt.bfloat16, kind="ExternalInput").ap()
    bwo = nc.dram_tensor("bwo", (DEPTH, KD, P), dt.float32, kind="ExternalInput").ap()
    w1 = nc.dram_tensor("w1", (DEPTH, D, HID), ffn_wdt, kind="ExternalInput").ap()
    b1 = nc.dram_tensor("b1", (DEPTH, HB, P), dt.float32, kind="ExternalInput").ap()
    s1 = nc.dram_tensor("s1", (DEPTH, HB, P), dt.float32, kind="ExternalInput").ap()
    w2 = nc.dram_tensor("w2", (DEPTH, HID, D), ffn_wdt, kind="ExternalInput").ap()
    b2 = nc.dram_tensor("b2", (DEPTH, KD, P), dt.float32, kind="ExternalInput").ap()
    s2 = nc.dram_tensor("s2", (DEPTH, KD, P), dt.float32, kind="ExternalInput").ap()
    wdec = nc.dram_tensor("wdec", (D, D), dt.bfloat16, kind="ExternalInput").ap()
    bdec = nc.dram_tensor("bdec", (KD, P), dt.float32, kind="ExternalInput").ap()
    y = nc.dram_tensor("y", (KD, P, TQ), dt.float16, kind="ExternalOutput").ap()

    with tile.TileContext(nc) as tc:
        from contextlib import ExitStack
        ctx = ExitStack()
        with ctx:
            const = ctx.enter_context(tc.tile_pool(name="const", bufs=1))
            xp = ctx.enter_context(tc.tile_pool(name="xp", bufs=1))
            xlnp = ctx.enter_context(tc.tile_pool(name="xlnp", bufs=1))
            qkvp = ctx.enter_context(tc.tile_pool(name="qkvp", bufs=1))
            vtp = ctx.enter_context(tc.tile_pool(name="vtp", bufs=1))
            ptp = ctx.enter_context(tc.tile_pool(name="ptp", bufs=2))
            aop = ctx.enter_context(tc.tile_pool(name="aop", bufs=1))
            hp = ctx.enter_context(tc.tile_pool(name="hp", bufs=1))
            yp = ctx.enter_context(tc.tile_pool(name="yp", bufs=1))
            wqp = ctx.enter_context(tc.tile_pool(name="wqp", bufs=2))
            wop = ctx.enter_context(tc.tile_pool(name="wop", bufs=1))
            w1p = ctx.enter_context(tc.tile_pool(name="w1p", bufs=4))
            w2p = ctx.enter_context(tc.tile_pool(name="w2p", bufs=3))
            scr = ctx.enter_context(tc.tile_pool(name="scr", bufs=2))
            xcp = ctx.enter_context(tc.tile_pool(name="xcp", bufs=2))
            statp = ctx.enter_context(tc.tile_pool(name="statp", bufs=1))
            biasp = ctx.enter_context(tc.tile_pool(name="biasp", bufs=1))

            drp = ctx.enter_context(tc.tile_pool(name="drp", bufs=1, space="DRAM"))
            # PSUM budget (8 banks): psA 3x2-bank pairs (6) + psC 1x2-bank pair (2)
            psA = ctx.enter_context(tc.tile_pool(name="psA", bufs=3, space="PSUM"))
            psC = ctx.enter_context(tc.tile_pool(name="psC", bufs=1, space="PSUM"))

            f32 = dt.float32
            bf = dt.bfloat16
            f16 = dt.float16
            AF = mybir.ActivationFunctionType
            ALU = mybir.AluOpType

            ones_kx1 = const.tile([P, 1], f32, tag="ones_kx1")
            nc.any.memset(ones_kx1[:], 1.0)
            ones_kx1b = const.tile([P, 1], bf, tag="ones_kx1b")
            nc.any.memset(ones_kx1b[:], 1.0)
            ones_kx1h = const.tile([P, 1], f16, tag="ones_kx1h")
            nc.any.memset(ones_kx1h[:], 1.0)
            ones_1xp_t = const.tile([1, P], f32, tag="ones_1xp")
            nc.any.memset(ones_1xp_t[:], 1.0)
            ones_1xp = ones_1xp_t[0:1, :]
            ones_bf_t = const.tile([1, P], bf, tag="ones_bf")
            nc.any.memset(ones_bf_t[:], 1.0)
            ones_bf = ones_bf_t[0:1, :]
            ident_bf = const.tile([P, P], bf, tag="ident_bf")
            make_identity(nc, ident_bf[:])
            eps_t = const.tile([1, 1], f32, tag="eps")
            nc.any.memset(eps_t[:], EPS)

            # persistent activations
            x = xp.tile([P, KD, T], f16, tag="x")
            xln = xlnp.tile([P, KD, T], qkv_wdt, tag="xln")
            aout = aop.tile([P, NH, TQ], bf, tag="aout")

            # per-core peer row indices for exchange readback
            pidx_t = const.tile([P, KD], dt.int32, tag="pidx")
            nc.sync.dma_start(pidx_t[:], pidx.rearrange("(a p) one -> p (a one)", p=P))

            # ---- PE warmup (runs during input DMA, releases HAM throttle) ----
            for wu in range(40):
                wt = psA.tile([P, 2, CH], bf, tag="a", name=f"wu{wu}")
                nc.tensor.transpose(wt[:, 0, 0:P], ident_bf[:], ident_bf[:])

            # ---- input DMA: x0^T chunks (own half first) ----
            for tb in (2, 3, 0, 1):
                for kt in range(KD):
                    nc.sync.dma_start(x[:, kt, tb * CH:(tb + 1) * CH],
                                      x0[kt, :, tb * CH:(tb + 1) * CH])

            def layer_norm(chunks, out_fn=None):
                """LN over feature dim of x for given chunk starts. Two passes:
                stats for ALL chunks first (PE runs gap-free), then broadcast +
                center/scale. Writes xln unless out_fn is given."""
                stb4 = statp.tile([1, 4, 2 * CH], bf, tag="stb4")
                for idx, cs in enumerate(chunks):
                    st = statp.tile([1, 4 * CH], f32, tag="stats")
                    sg = lambda i: st[0:1, i * CH:(i + 1) * CH]
                    ps = psA.tile([P, 2, CH], f32, tag="a")
                    for kt in range(KD):
                        nc.tensor.matmul(ps[0:1, 0, :], ones_kx1h[:], x[:, kt, cs:cs + CH],
                                         start=(kt == 0), stop=(kt == KD - 1))
                    for kt in range(KD):
                        sq = scr.tile([P, CH], bf, tag="scrb")
                        nc.scalar.activation(sq[:], x[:, kt, cs:cs + CH], AF.Square)
                        nc.tensor.matmul(ps[0:1, 1, :], ones_kx1b[:], sq[:],
                                         start=(kt == 0), stop=(kt == KD - 1))
                    nc.vector.tensor_scalar_mul(st[0:1, 2 * CH:4 * CH],
                                                 ps[0:1, 0:2, :], 1.0 / D)   # [mu, E2]
                    nc.vector.tensor_mul(sg(0), sg(2), sg(2))                # mu^2
                    nc.vector.tensor_sub(sg(1), sg(3), sg(0))                # var
                    nc.scalar.activation(sg(0), sg(1), AF.Sqrt,
                                         bias=eps_t[0:1, 0:1])
                    nc.vector.reciprocal_approx_fast(sg(3), sg(0))           # r (seg3)
                    nc.vector.tensor_copy(stb4[0:1, idx, :], st[0:1, 2 * CH:4 * CH])
                    pmr = psC.tile([P, 2, CH], f32, tag="c")
                    nc.tensor.matmul(pmr[:, 0, :], ones_bf, stb4[0:1, idx, 0:CH],
                                     start=True, stop=True)
                    nc.tensor.matmul(pmr[:, 1, :], ones_bf, stb4[0:1, idx, CH:2 * CH],
                                     start=True, stop=True)
                    pms = xcp.tile([P, 2, CH], bf, tag="pms")
                    nc.scalar.activation(pms[:, 0:2, :], pmr[:, 0:2, :], AF.Copy)
                    for kt in range(KD):
                        xc = xcp.tile([P, CH], bf, tag="xc")
                        eng = nc.gpsimd if kt % 3 == 2 else nc.vector
                        eng.tensor_sub(xc[:], x[:, kt, cs:cs + CH], pms[:, 0, :])
                        if out_fn is None:
                            eng.tensor_mul(xln[:, kt, cs:cs + CH], xc[:],
                                           pms[:, 1, :])
                        else:
                            out_fn(kt, cs, xc, pms[:, 1, :], eng)

            CC = [None, None]  # collective outs per chunk

            for l in range(DEPTH):
                # ---------- LN1 ----------
                if l == 0:
                    layer_norm([Q0, Q0 + CH, 0, CH])
                else:
                    # readback of peer half (gpsimd queue; waits on collectives)
                    for ci in range(2):
                        for kt in range(KD):
                            nc.gpsimd.indirect_dma_start(
                                out=x[:, kt, ci * CH:(ci + 1) * CH], out_offset=None,
                                in_=CC[ci][:],
                                in_offset=bass.IndirectOffsetOnAxis(
                                    ap=pidx_t[:, kt:kt + 1], axis=0),
                            )
                    layer_norm([Q0, Q0 + CH, 0, CH])

                # per-layer bias/scale tiles
                bq = biasp.tile([P, NH * 3], f32, tag="bq")
                nc.sync.dma_start(bq[:], bqkv[l].rearrange("a p -> p a"))
                sq_t = biasp.tile([P, NH * 3], f32, tag="sq")
                nc.sync.dma_start(sq_t[:], sqkv[l].rearrange("a p -> p a"))
                bo_t = biasp.tile([P, KD], f32, tag="bo")
                nc.sync.dma_start(bo_t[:], bwo[l].rearrange("a p -> p a"))
                b1_t = biasp.tile([P, HB], f32, tag="b1")
                nc.sync.dma_start(b1_t[:], b1[l].rearrange("a p -> p a"))
                s1_t = biasp.tile([P, HB], f32, tag="s1")
                nc.sync.dma_start(s1_t[:], s1[l].rearrange("a p -> p a"))
                b2_t = biasp.tile([P, KD], f32, tag="b2")
                nc.sync.dma_start(b2_t[:], b2[l].rearrange("a p -> p a"))
                s2_t = biasp.tile([P, KD], f32, tag="s2")
                nc.sync.dma_start(s2_t[:], s2[l].rearrange("a p -> p a"))

                wot = wop.tile([P, NH, D], bf, tag="wo")
                nc.sync.dma_start(wot[:], wo[l].rearrange("(kb p) c -> p kb c", p=P))

                # chunk-pair order: own half first
                kv_pairs = [Q0, 0]
                halves = [1, 0]

                # ---------- attention, head-by-head ----------
                for h in range(NH):
                    wqt = wqp.tile([P, KD, 3 * P], qkv_wdt, tag="wq")
                    for g in range(2):
                        nc.sync.dma_start(
                            wqt[:, 3 * g:3 * g + 3, :],
                            wqkv[l, 3 * g * P:(3 * g + 3) * P,
                                 h * 3 * P:(h + 1) * 3 * P].rearrange(
                                "(kt p) c -> p kt c", p=P))
                    qkvh = qkvp.tile([P, 3, T], bf, tag="qkvh")

                    def qkv_chain(m, c0, c1, bank_ps, bank):
                        """one 512-col chain for q/k/v block m into psum bank."""
                        if USE_FP8_QKV:
                            for j in range(KD // 2):
                                nc.tensor.matmul(
                                    bank_ps[:, bank, :],
                                    wqt[:, 2 * j:2 * j + 2, m * P:(m + 1) * P],
                                    xln[:, 2 * j:2 * j + 2, c0:c1],
                                    start=(j == 0), stop=(j == KD // 2 - 1),
                                    perf_mode=mybir.MatmulPerfMode.DoubleRow)
                        else:
                            for kt in range(KD):
                                nc.tensor.matmul(bank_ps[:, bank, :],
                                                 wqt[:, kt, m * P:(m + 1) * P],
                                                 xln[:, kt, c0:c1],
                                                 start=(kt == 0), stop=(kt == KD - 1))

                    for m in range(3):
                        pairs = [Q0] if m == 0 else kv_pairs
                        for p0 in pairs:
                            ps = psA.tile([P, 2, CH], f32, tag="a")
                            qkv_chain(m, p0, p0 + CH, ps, 0)
                            qkv_chain(m, p0 + CH, p0 + 2 * CH, ps, 1)
                            c = h * 3 + m
                            nc.vector.tensor_scalar(
                                qkvh[:, m, p0:p0 + 2 * CH], ps[:, 0:2, :],
                                sq_t[:, c:c + 1], bq[:, c:c + 1],
                                ALU.mult, ALU.add)

                    # transpose V (and its ones-row) -> vaug [T, 128]
                    vaug = vtp.tile([P, T // P, P], bf, tag="vaug")
                    for half in halves:
                        for tb in range(half * 8, half * 8 + 8):
                            pst = psA.tile([P, 2, CH], bf, tag="a")
                            nc.tensor.transpose(pst[:, 0, 0:P],
                                                qkvh[:, 2, tb * P:(tb + 1) * P],
                                                ident_bf[:])
                            nc.vector.tensor_copy(vaug[:, tb, :], pst[:, 0, 0:P])
                    # scores^T -> exp -> PV, per query chunk
                    for cs in (Q0, Q0 + CH):
                        pvc = psC.tile([P, 2, CH], f32, tag="c")
                        pv = pvc[:, 0, :]
                        nmm = 0
                        for half in halves:
                            pt = ptp.tile([P, 8, CH], bf, tag="pt")
                            for tp in range(4):
                                tb = half * 8 + 2 * tp
                                sc = psA.tile([P, 2, CH], f32, tag="a")
                                for k2 in range(2):
                                    nc.tensor.matmul(
                                        sc[:, k2, :],
                                        qkvh[:, 1, (tb + k2) * P:(tb + k2 + 1) * P],
                                        qkvh[:, 0, cs:cs + CH],
                                        start=True, stop=True)
                                nc.scalar.activation(pt[:, 2 * tp:2 * tp + 2, :],
                                                     sc[:, 0:2, :], AF.Exp)
                                for k2 in range(2):
                                    nc.tensor.matmul(pv, vaug[:, tb + k2, :],
                                                     pt[:, 2 * tp + k2, :],
                                                     start=(nmm == 0),
                                                     stop=(nmm == T // P - 1))
                                    nmm += 1
                        # normalize by denominator (row 96 of pv)
                        dn = scr.tile([1, CH], bf, tag="scrb")
                        nc.vector.tensor_copy(dn[:], pv[DH:DH + 1, :])
                        nc.tensor.matmul(pvc[:, 1, :], ones_bf, dn[:],
                                         start=True, stop=True)
                        rc = scr.tile([P, CH], f32, tag="scr")
                        nc.vector.reciprocal_approx_fast(rc[:], pvc[:, 1, :])
                        nc.vector.tensor_mul(aout[:, h, cs - Q0:cs - Q0 + CH], pv, rc[:])

                # ---------- Wo + residual ----------
                for cs in (Q0, Q0 + CH):
                    for mp in range(KD // 2):
                        ps = psA.tile([P, 2, CH], f32, tag="a")
                        for bank in range(2):
                            m = 2 * mp + bank
                            for kb in range(NH):
                                nc.tensor.matmul(ps[:, bank, :],
                                                 wot[:, kb, m * P:(m + 1) * P],
                                                 aout[:, kb, cs - Q0:cs - Q0 + CH],
                                                 start=(kb == 0), stop=(kb == NH - 1))
                        for bank in range(2):
                            m = 2 * mp + bank
                            t = scr.tile([P, CH], f32, tag="scr")
                            nc.scalar.activation(t[:], ps[:, bank, :], AF.Identity,
                                                 bias=bo_t[:, m:m + 1])
                            nc.vector.tensor_add(x[:, m, cs:cs + CH], x[:, m, cs:cs + CH],
                                                 t[:])

                # ---------- LN2 + FFN + residual ----------
                layer_norm([Q0, Q0 + CH])
                for ci, cs in enumerate((Q0, Q0 + CH)):
                    ht = hp.tile([P, HB, CH], ffn_wdt, tag="h")
                    for hp2 in range(HB // 2):
                        w1t = w1p.tile([P, KD, 2 * P], ffn_wdt, tag="w1")
                        for g in range(2):
                            nc.sync.dma_start(
                                w1t[:, 3 * g:3 * g + 3, :],
                                w1[l, 3 * g * P:(3 * g + 3) * P,
                                   hp2 * 2 * P:(hp2 + 1) * 2 * P].rearrange(
                                    "(kt p) c -> p kt c", p=P))
                        ph = psA.tile([P, 2, CH], f32, tag="a")
                        for bank in range(2):
                            hb = 2 * hp2 + bank
                            if USE_FP8_FFN:
                                for j in range(KD // 2):
                                    nc.tensor.matmul(
                                        ph[:, bank, :],
                                        w1t[:, 2 * j:2 * j + 2, bank * P:(bank + 1) * P],
                                        xln[:, 2 * j:2 * j + 2, cs:cs + CH],
                                        start=(j == 0), stop=(j == KD // 2 - 1),
                                        perf_mode=mybir.MatmulPerfMode.DoubleRow)
                            else:
                                for kt in range(KD):
                                    nc.tensor.matmul(ph[:, bank, :],
                                                     w1t[:, kt, bank * P:(bank + 1) * P],
                                                     xln[:, kt, cs:cs + CH],
                                                     start=(kt == 0), stop=(kt == KD - 1))
                        for bank in range(2):
                            hb = 2 * hp2 + bank
                            nc.scalar.activation(ht[:, hb, :], ph[:, bank, :], AF.Gelu,
                                                 bias=b1_t[:, hb:hb + 1],
                                                 scale=s1_t[:, hb:hb + 1])
                    # W2: 6 output blocks in parallel chains (2 psA pairs + 2 psD)
                    pa0 = psA.tile([P, 2, CH], f32, tag="a")
                    pa1 = psA.tile([P, 2, CH], f32, tag="a")
                    pa2 = psC.tile([P, 2, CH], f32, tag="c")
                    chains = [pa0[:, 0, :], pa0[:, 1, :], pa1[:, 0, :], pa1[:, 1, :],
                              pa2[:, 0, :], pa2[:, 1, :]]
                    if USE_FP8_FFN:
                        for j in range(HB // 2):
                            w2t = w2p.tile([P, 2, D], ffn_wdt, tag="w2")
                            nc.sync.dma_start(
                                w2t[:], w2[l, 2 * j * P:(2 * j + 2) * P, :].rearrange(
                                    "(two p) d -> p two d", p=P))
                            for m in range(KD):
                                nc.tensor.matmul(
                                    chains[m], w2t[:, :, m * P:(m + 1) * P],
                                    ht[:, 2 * j:2 * j + 2, :],
                                    start=(j == 0), stop=(j == HB // 2 - 1),
                                    perf_mode=mybir.MatmulPerfMode.DoubleRow)
                    else:
                        for kb in range(HB):
                            w2t = w2p.tile([P, 2, D], ffn_wdt, tag="w2")
                            nc.sync.dma_start(
                                w2t[:, 0, :], w2[l, kb * P:(kb + 1) * P, :])
                            for m in range(KD):
                                nc.tensor.matmul(chains[m], w2t[:, 0, m * P:(m + 1) * P],
                                                 ht[:, kb, :],
                                                 start=(kb == 0), stop=(kb == HB - 1))
                    for m in range(KD):
                        t = scr.tile([P, CH], f32, tag="scr")
                        nc.vector.tensor_scalar(t[:], chains[m],
                                                s2_t[:, m:m + 1], b2_t[:, m:m + 1],
                                                ALU.mult, ALU.add)
                        nc.vector.tensor_add(x[:, m, cs:cs + CH],
                                             x[:, m, cs:cs + CH], t[:])

                    if l == 0:
                        # ship this chunk of x1 own-half to the peer, feature-major
                        cc_in = drp.tile([KD * P, CH], f16, tag=f"cci{ci}",
                                         name=f"cci{ci}")
                        cc_out = drp.tile([2 * KD * P, CH], f16, tag=f"cco{ci}",
                                          name=f"cco{ci}")
                        CC[ci] = cc_out
                        for kt in range(KD):
                            nc.sync.dma_start(cc_in[kt * P:(kt + 1) * P, :],
                                              x[:, kt, cs:cs + CH])
                        nc.gpsimd.collective_compute(
                            "AllGather",
                            mybir.AluOpType.bypass,
                            replica_groups=[[0, 1], [2, 3], [4, 5], [6, 7]],
                            ins=[cc_in.opt()],
                            outs=[cc_out.opt()],
                        )

            # ---------- final LN + decoder head ----------
            # final LN writes bf16 into the (now free) aout tile
            def to_aout(kt, cs, xc, pr_ap, eng):
                c0 = cs - Q0
                eng.tensor_mul(aout[:, kt, c0:c0 + CH], xc[:], pr_ap)

            layer_norm([Q0, Q0 + CH], to_aout)

            wdect = const.tile([P, KD, 6 * P], bf, tag="wdec")
            nc.sync.dma_start(wdect[:], wdec.rearrange("(kt p) c -> p kt c", p=P))
            bd_t = biasp.tile([P, KD], f32, tag="bd")
            nc.sync.dma_start(bd_t[:], bdec.rearrange("a p -> p a"))
            for ci in range(2):
                yT = yp.tile([P, KD, CH], f16, tag="yT")
                for mp in range(KD // 2):
                    ps = psA.tile([P, 2, CH], f32, tag="a")
                    for bank in range(2):
                        m = 2 * mp + bank
                        for kt in range(KD):
                            nc.tensor.matmul(ps[:, bank, :],
                                             wdect[:, kt, m * P:(m + 1) * P],
                                             aout[:, kt, ci * CH:(ci + 1) * CH],
                                             start=(kt == 0), stop=(kt == KD - 1))
                    for bank in range(2):
                        m = 2 * mp + bank
                        nc.scalar.activation(yT[:, m, :], ps[:, bank, :], AF.Identity,
                                             bias=bd_t[:, m:m + 1])
                for kt in range(KD):
                    nc.sync.dma_start(y[kt, :, ci * CH:(ci + 1) * CH], yT[:, kt, :])

    nc.compile()
    return nc


def _quant_rows(w):
    """Per-output-row e4m3 quantization. w: [out, in] -> (wq fp8, scale[out])"""
    amax = np.maximum(np.abs(w).max(axis=1), 1e-20)
    s = 192.0 / amax
    wq = (w * s[:, None]).astype(F8)
    return wq, (1.0 / s).astype(np.float32)


def _prep_weights(inputs):
    """Host-side weight folding/packing. Returns dict of shared arrays."""
    g1, be1 = inputs["gamma1"], inputs["beta1"]
    g2, be2 = inputs["gamma2"], inputs["beta2"]
    Wqkv, bqkv = inputs["Wqkv"], inputs["bqkv"]
    Wo, bo = inputs["Wo"], inputs["bo"]
    W1, b1 = inputs["W1"], inputs["b1"]
    W2, b2 = inputs["W2"], inputs["b2"]
    gn, gb = inputs["gn"], inputs["gb"]
    Wdec, bdec = inputs["Wdec"], inputs["bdec"]

    qdt = F8 if USE_FP8_QKV else BF16
    fdt = F8 if USE_FP8_FFN else BF16
    wqkv_a = np.zeros((DEPTH, D, NH * 3 * P), qdt)
    bqkv_a = np.zeros((DEPTH, NH * 3, P), np.float32)
    sqkv_a = np.ones((DEPTH, NH * 3, P), np.float32)
    wo_a = np.zeros((DEPTH, NH * P, D), np.float32)
    bwo_a = np.zeros((DEPTH, KD, P), np.float32)
    w1_a = np.zeros((DEPTH, D, HID), fdt)
    b1_a = np.zeros((DEPTH, HB, P), np.float32)
    s1_a = np.ones((DEPTH, HB, P), np.float32)
    w2_a = np.zeros((DEPTH, HID, D), fdt)
    b2_a = np.zeros((DEPTH, KD, P), np.float32)
    s2_a = np.ones((DEPTH, KD, P), np.float32)
    scale = 1.0 / np.sqrt(DH)
    for l in range(DEPTH):
        Wp = Wqkv[l] * g1[l][None, :]                  # fold gamma1
        bp = bqkv[l] + Wqkv[l] @ be1[l]                # fold beta1
        Wp = Wp.copy()
        bp = bp.copy()
        Wp[:D] *= scale                                # fold 1/sqrt(dh) into Q
        bp[:D] *= scale
        if USE_FP8_QKV:
            Wpq, Wps = _quant_rows(Wp)
        else:
            Wpq, Wps = Wp.astype(BF16), np.ones(3 * D, np.float32)
        for h in range(NH):
            for c in range(3):                         # q,k,v
                rows = slice(c * D + h * DH, c * D + (h + 1) * DH)
                wqkv_a[l, :, (h * 3 + c) * P:(h * 3 + c) * P + DH] = Wpq[rows].T
                bqkv_a[l, h * 3 + c, :DH] = bp[rows]
                sqkv_a[l, h * 3 + c, :DH] = Wps[rows]
            bqkv_a[l, h * 3 + 2, DH] = 1.0             # ones-row -> denominators
            wo_a[l, h * P:h * P + DH, :] = Wo[l][:, h * DH:(h + 1) * DH].T
        bwo_a[l] = bo[l].reshape(KD, P)
        W1f = W1[l] * g2[l][None, :]
        b1f = b1[l] + W1[l] @ be2[l]
        if USE_FP8_FFN:
            W1q, W1s = _quant_rows(W1f)
            W2q, W2s = _quant_rows(W2[l])
        else:
            W1q, W1s = W1f.astype(BF16), np.ones(HID, np.float32)
            W2q, W2s = W2[l].astype(BF16), np.ones(D, np.float32)
        w1_a[l] = W1q.T
        b1_a[l] = b1f.reshape(HB, P)
        s1_a[l] = W1s.reshape(HB, P)
        w2_a[l] = W2q.T
        b2_a[l] = b2[l].reshape(KD, P)
        s2_a[l] = W2s.reshape(KD, P)
    wdec_a = (Wdec * gn[None, :]).T
    bdec_a = (bdec + Wdec @ gb).reshape(KD, P)
    return {
        "wqkv": wqkv_a, "bqkv": bqkv_a, "sqkv": sqkv_a,
        "wo": wo_a.astype(BF16), "bwo": bwo_a,
        "w1": w1_a, "b1": b1_a, "s1": s1_a,
        "w2": w2_a, "b2": b2_a, "s2": s2_a,
        "wdec": wdec_a.astype(BF16), "bdec": bdec_a,
    }


def kernel(**inputs):
    from concourse.bass_utils import run_bass_kernel_spmd

    inputs = {k: np.asarray(v) for k, v in inputs.items()}
    if "nc" not in _cache:
        _cache["nc"] = _build()
    nc = _cache["nc"]

    shared = _prep_weights(inputs)
    mask = inputs["mask"]
    vt = inputs["visible_tokens"].astype(np.float32)
    mt = inputs["mask_token"].astype(np.float32)

    # host-side scatter: x0[b, t] = vt[b, idx] if mask else mask_token
    nv = np.clip(np.cumsum(mask.astype(np.int64), axis=1) - 1, 0, N_VIS - 1)
    gathered = np.take_along_axis(vt, nv[..., None], axis=1)
    x0_full = np.where(mask[..., None], gathered, mt[None, None, :])  # (B,T,D)

    in_maps = []
    for core in range(8):
        b, s = core // 2, core % 2
        if s == 0:
            perm = np.concatenate([np.arange(TQ, T), np.arange(0, TQ)])
        else:
            perm = np.arange(T)
        x0p = np.ascontiguousarray(
            x0_full[b][perm].T.astype(np.float16).reshape(KD, P, T))
        peer = 1 - s
        m = dict(shared)
        m["x0"] = x0p
        m["pidx"] = (peer * D + np.arange(D, dtype=np.int32))[:, None].copy()
        in_maps.append(m)

    res = run_bass_kernel_spmd(nc, in_maps, core_ids=list(range(8)),
                               **_cache.get("run_kwargs", {}))
    _cache["last_results"] = res

    out = np.zeros((B, T, D), np.float32)
    for core in range(8):
        b, s = core // 2, core % 2
        yv = res.results[core]["y"].reshape(D, TQ).astype(np.float32)
        out[b, s * TQ:(s + 1) * TQ] = yv.T
    return out


if __name__ == "__main__":
    rng = np.random.default_rng(0)
    print("building...")
    _build()
    print("built ok")


# revision 44
# speedup vs baseline: 1.0156x; 1.0156x over previous
"""Trainium2 Bass kernel for nn_DecoderHead (MAE-style decoder head).

Strategy (8 NeuronCores): data-parallel over batch B=4 x 2-way token split
per batch. Cores sharing a batch both compute layer 0 K/V for all 2048
tokens, then exchange x1 halves (feature-major, via AllGather + indirect
readback) so layer 1 + head run on their own 1024-token half.

On-device layout is feature-major (x^T: [D, T] with D on partitions).
Input scatter and output transpose are done host-side; the device input is
x0^T pre-permuted per core. Heads padded 96->128; a ones-row injected in V
(via bias) makes PV emit softmax denominators for free. LN gamma/beta are
folded into adjacent weights host-side. All GEMMs run in bf16 (fp8e4
DoubleRow paths exist behind USE_FP8_* flags but e4m3 noise exceeds the
harness tolerance); PSUM is organized as 2-bank pair tiles (3+1 rotation)
so evictions are 1024 wide and the PE stays deep in its pipeline.
"""

import sys
import numpy as np

sys.path.insert(0, "/opt/trn_rl_repo")

import ml_dtypes

P = 128
B = 4
N_VIS = 512
T = 2048          # N_TOT
D = 768
KD = D // P       # 6
NH = 8
DH = 96
HID = 3072
HB = HID // P     # 24
DEPTH = 2
TQ = 1024         # own-half tokens per core
CH = 512          # token chunk
Q0 = T - TQ
EPS = 1e-5

BF16 = ml_dtypes.bfloat16
F8 = ml_dtypes.float8_e4m3

USE_FP8_QKV = False
USE_FP8_FFN = False

_cache = {}


def _build():
    import concourse.bass as bass
    import concourse.mybir as mybir
    import concourse.tile as tile
    from concourse import bacc
    from concourse.masks import make_identity

    dt = mybir.dt
    nc = bacc.Bacc("TRN2", target_bir_lowering=False, debug=False, num_devices=8)

    f8 = dt.float8e4
    qkv_wdt = f8 if USE_FP8_QKV else dt.bfloat16
    ffn_wdt = f8 if USE_FP8_FFN else dt.bfloat16

    x0 = nc.dram_tensor("x0", (KD, P, T), dt.float16, kind="ExternalInput").ap()
    pidx = nc.dram_tensor("pidx", (P, KD), dt.int32, kind="ExternalInput").ap()
    wqkv = nc.dram_tensor("wqkv", (DEPTH, D, NH * 3 * P), qkv_wdt, kind="ExternalInput").ap()
    bqkv = nc.dram_tensor("bqkv", (DEPTH, P, NH * 3), dt.float32, kind="ExternalInput").ap()
    sqkv = nc.dram_tensor("sqkv", (DEPTH, P, NH * 3), dt.float32, kind="ExternalInput").ap()
    wo = nc.dram_tensor("wo", (DEPTH, NH * P, D), dt.bfloat16, kind="ExternalInput").ap()
    bwo = nc.dram_tensor("bwo", (DEPTH, P, KD), dt.float32, kind="ExternalInput").ap()
    w1 = nc.dram_tensor("w1", (DEPTH, D, HID), ffn_wdt, kind="ExternalInput").ap()
    b1 = nc.dram_tensor("b1", (DEPTH, P, HB), dt.float32, kind="ExternalInput").ap()
    s1 = nc.dram_tensor("s1", (DEPTH, P, HB), dt.float32, kind="ExternalInput").ap()
    w2 = nc.dram_tensor("w2", (DEPTH, HID, D), ffn_wdt, kind="ExternalInput").ap()
    b2 = nc.dram_tensor("b2", (DEPTH, P, KD), dt.float32, kind="ExternalInput").ap()
    s2 = nc.dram_tensor("s2", (DEPTH, P, KD), dt.float32, kind="ExternalInput").ap()
    wdec = nc.dram_tensor("wdec", (D, D), dt.bfloat16, kind="ExternalInput").ap()
    bdec = nc.dram_tensor("bdec", (P, KD), dt.float32, kind="ExternalInput").ap()
    y = nc.dram_tensor("y", (KD, P, TQ), dt.float16, kind="ExternalOutput").ap()

    with tile.TileContext(nc) as tc:
        from contextlib import ExitStack
        ctx = ExitStack()
        with ctx:
            const = ctx.enter_context(tc.tile_pool(name="const", bufs=1))
            xp = ctx.enter_context(tc.tile_pool(name="xp", bufs=1))
            xlnp = ctx.enter_context(tc.tile_pool(name="xlnp", bufs=1))
            qkvp = ctx.enter_context(tc.tile_pool(name="qkvp", bufs=1))
            vtp = ctx.enter_context(tc.tile_pool(name="vtp", bufs=1))
            ptp = ctx.enter_context(tc.tile_pool(name="ptp", bufs=2))
            aop = ctx.enter_context(tc.tile_pool(name="aop", bufs=1))
            hp = ctx.enter_context(tc.tile_pool(name="hp", bufs=1))
            yp = ctx.enter_context(tc.tile_pool(name="yp", bufs=1))
            wqp = ctx.enter_context(tc.tile_pool(name="wqp", bufs=2))
            wop = ctx.enter_context(tc.tile_pool(name="wop", bufs=1))
            w1p = ctx.enter_context(tc.tile_pool(name="w1p", bufs=4))
            w2p = ctx.enter_context(tc.tile_pool(name="w2p", bufs=3))
            scr = ctx.enter_context(tc.tile_pool(name="scr", bufs=2))
            xcp = ctx.enter_context(tc.tile_pool(name="xcp", bufs=2))
            statp = ctx.enter_context(tc.tile_pool(name="statp", bufs=1))
            biasp = ctx.enter_context(tc.tile_pool(name="biasp", bufs=1))

            drp = ctx.enter_context(tc.tile_pool(name="drp", bufs=1, space="DRAM"))
            # PSUM budget (8 banks): psA 3x2-bank pairs (6) + psC 1x2-bank pair (2)
            psA = ctx.enter_context(tc.tile_pool(name="psA", bufs=3, space="PSUM"))
            psC = ctx.enter_context(tc.tile_pool(name="psC", bufs=1, space="PSUM"))

            f32 = dt.float32
            bf = dt.bfloat16
            f16 = dt.float16
            AF = mybir.ActivationFunctionType
            ALU = mybir.AluOpType

            ones_kx1 = const.tile([P, 1], f32, tag="ones_kx1")
            nc.any.memset(ones_kx1[:], 1.0)
            ones_kx1b = const.tile([P, 1], bf, tag="ones_kx1b")
            nc.any.memset(ones_kx1b[:], 1.0)
            ones_kx1h = const.tile([P, 1], f16, tag="ones_kx1h")
            nc.any.memset(ones_kx1h[:], 1.0)
            ones_1xp_t = const.tile([1, P], f32, tag="ones_1xp")
            nc.any.memset(ones_1xp_t[:], 1.0)
            ones_1xp = ones_1xp_t[0:1, :]
            ones_bf_t = const.tile([1, P], bf, tag="ones_bf")
            nc.any.memset(ones_bf_t[:], 1.0)
            ones_bf = ones_bf_t[0:1, :]
            ident_bf = const.tile([P, P], bf, tag="ident_bf")
            make_identity(nc, ident_bf[:])
            eps_t = const.tile([1, 1], f32, tag="eps")
            nc.any.memset(eps_t[:], EPS)

            # persistent activations
            x = xp.tile([P, KD, T], f16, tag="x")
            xln = xlnp.tile([P, KD, T], qkv_wdt, tag="xln")
            aout = aop.tile([P, NH, TQ], bf, tag="aout")

            # per-core peer row indices for exchange readback
            pidx_t = const.tile([P, KD], dt.int32, tag="pidx")
            nc.sync.dma_start(pidx_t[:], pidx[:])

            # ---- PE warmup (runs during input DMA, releases HAM throttle) ----
            for wu in range(40):
                wt = psA.tile([P, 2, CH], bf, tag="a", name=f"wu{wu}")
                nc.tensor.transpose(wt[:, 0, 0:P], ident_bf[:], ident_bf[:])

            # ---- input DMA: x0^T chunks (own half first) ----
            for tb in (2, 3, 0, 1):
                for kt in range(KD):
                    nc.sync.dma_start(x[:, kt, tb * CH:(tb + 1) * CH],
                                      x0[kt, :, tb * CH:(tb + 1) * CH])

            def layer_norm(chunks, out_fn=None):
                """LN over feature dim of x for given chunk starts. Two passes:
                stats for ALL chunks first (PE runs gap-free), then broadcast +
                center/scale. Writes xln unless out_fn is given."""
                stb4 = statp.tile([1, 4, 2 * CH], bf, tag="stb4")
                for idx, cs in enumerate(chunks):
                    st = statp.tile([1, 4 * CH], f32, tag="stats")
                    sg = lambda i: st[0:1, i * CH:(i + 1) * CH]
                    ps = psA.tile([P, 2, CH], f32, tag="a")
                    for kt in range(KD):
                        nc.tensor.matmul(ps[0:1, 0, :], ones_kx1h[:], x[:, kt, cs:cs + CH],
                                         start=(kt == 0), stop=(kt == KD - 1))
                    for kt in range(KD):
                        sq = scr.tile([P, CH], bf, tag="scrb")
                        nc.scalar.activation(sq[:], x[:, kt, cs:cs + CH], AF.Square)
                        nc.tensor.matmul(ps[0:1, 1, :], ones_kx1b[:], sq[:],
                                         start=(kt == 0), stop=(kt == KD - 1))
                    nc.vector.tensor_scalar_mul(st[0:1, 2 * CH:4 * CH],
                                                 ps[0:1, 0:2, :], 1.0 / D)   # [mu, E2]
                    nc.vector.tensor_mul(sg(0), sg(2), sg(2))                # mu^2
                    nc.vector.tensor_sub(sg(1), sg(3), sg(0))                # var
                    nc.scalar.activation(sg(0), sg(1), AF.Sqrt,
                                         bias=eps_t[0:1, 0:1])
                    nc.vector.reciprocal_approx_fast(sg(3), sg(0))           # r (seg3)
                    nc.vector.tensor_copy(stb4[0:1, idx, :], st[0:1, 2 * CH:4 * CH])
                    pmr = psC.tile([P, 2, CH], f32, tag="c")
                    nc.tensor.matmul(pmr[:, 0, :], ones_bf, stb4[0:1, idx, 0:CH],
                                     start=True, stop=True)
                    nc.tensor.matmul(pmr[:, 1, :], ones_bf, stb4[0:1, idx, CH:2 * CH],
                                     start=True, stop=True)
                    pms = xcp.tile([P, 2, CH], bf, tag="pms")
                    nc.scalar.activation(pms[:, 0:2, :], pmr[:, 0:2, :], AF.Copy)
                    for kt in range(KD):
                        xc = xcp.tile([P, CH], bf, tag="xc")
                        eng = nc.gpsimd if kt % 3 == 2 else nc.vector
                        eng.tensor_sub(xc[:], x[:, kt, cs:cs + CH], pms[:, 0, :])
                        if out_fn is None:
                            eng.tensor_mul(xln[:, kt, cs:cs + CH], xc[:],
                                           pms[:, 1, :])
                        else:
                            out_fn(kt, cs, xc, pms[:, 1, :], eng)

            CC = [None, None]  # collective outs per chunk

            for l in range(DEPTH):
                # ---------- LN1 ----------
                if l == 0:
                    layer_norm([Q0, Q0 + CH, 0, CH])
                else:
                    # readback of peer half (gpsimd queue; waits on collectives)
                    for ci in range(2):
                        for kt in range(KD):
                            nc.gpsimd.indirect_dma_start(
                                out=x[:, kt, ci * CH:(ci + 1) * CH], out_offset=None,
                                in_=CC[ci][:],
                                in_offset=bass.IndirectOffsetOnAxis(
                                    ap=pidx_t[:, kt:kt + 1], axis=0),
                            )
                    layer_norm([Q0, Q0 + CH, 0, CH])

                # per-layer bias/scale tiles
                bq = biasp.tile([P, NH * 3], f32, tag="bq")
                nc.sync.dma_start(bq[:], bqkv[l])
                sq_t = biasp.tile([P, NH * 3], f32, tag="sq")
                nc.sync.dma_start(sq_t[:], sqkv[l])
                bo_t = biasp.tile([P, KD], f32, tag="bo")
                nc.sync.dma_start(bo_t[:], bwo[l])
                b1_t = biasp.tile([P, HB], f32, tag="b1")
                nc.sync.dma_start(b1_t[:], b1[l])
                s1_t = biasp.tile([P, HB], f32, tag="s1")
                nc.sync.dma_start(s1_t[:], s1[l])
                b2_t = biasp.tile([P, KD], f32, tag="b2")
                nc.sync.dma_start(b2_t[:], b2[l])
                s2_t = biasp.tile([P, KD], f32, tag="s2")
                nc.sync.dma_start(s2_t[:], s2[l])

                wot = wop.tile([P, NH, D], bf, tag="wo")
                nc.sync.dma_start(wot[:], wo[l].rearrange("(kb p) c -> p kb c", p=P))

                # chunk-pair order: own half first
                kv_pairs = [Q0, 0]
                halves = [1, 0]

                # ---------- attention, head-by-head ----------
                for h in range(NH):
                    wqt = wqp.tile([P, KD, 3 * P], qkv_wdt, tag="wq")
                    nc.sync.dma_start(
                        wqt[:], wqkv[l, :, h * 3 * P:(h + 1) * 3 * P].rearrange(
                            "(kt p) c -> p kt c", p=P))
                    qkvh = qkvp.tile([P, 3, T], bf, tag="qkvh")

                    def qkv_chain(m, c0, c1, bank_ps, bank):
                        """one 512-col chain for q/k/v block m into psum bank."""
                        if USE_FP8_QKV:
                            for j in range(KD // 2):
                                nc.tensor.matmul(
                                    bank_ps[:, bank, :],
                                    wqt[:, 2 * j:2 * j + 2, m * P:(m + 1) * P],
                                    xln[:, 2 * j:2 * j + 2, c0:c1],
                                    start=(j == 0), stop=(j == KD // 2 - 1),
                                    perf_mode=mybir.MatmulPerfMode.DoubleRow)
                        else:
                            for kt in range(KD):
                                nc.tensor.matmul(bank_ps[:, bank, :],
                                                 wqt[:, kt, m * P:(m + 1) * P],
                                                 xln[:, kt, c0:c1],
                                                 start=(kt == 0), stop=(kt == KD - 1))

                    for m in range(3):
                        pairs = [Q0] if m == 0 else kv_pairs
                        for p0 in pairs:
                            ps = psA.tile([P, 2, CH], f32, tag="a")
                            qkv_chain(m, p0, p0 + CH, ps, 0)
                            qkv_chain(m, p0 + CH, p0 + 2 * CH, ps, 1)
                            c = h * 3 + m
                            nc.vector.tensor_scalar(
                                qkvh[:, m, p0:p0 + 2 * CH], ps[:, 0:2, :],
                                sq_t[:, c:c + 1], bq[:, c:c + 1],
                                ALU.mult, ALU.add)

                    # transpose V (and its ones-row) -> vaug [T, 128]
                    vaug = vtp.tile([P, T // P, P], bf, tag="vaug")
                    for half in halves:
                        for tb in range(half * 8, half * 8 + 8):
                            pst = psA.tile([P, 2, CH], bf, tag="a")
                            nc.tensor.transpose(pst[:, 0, 0:P],
                                                qkvh[:, 2, tb * P:(tb + 1) * P],
                                                ident_bf[:])
                            nc.vector.tensor_copy(vaug[:, tb, :], pst[:, 0, 0:P])
                    # scores^T -> exp -> PV, per query chunk
                    for cs in (Q0, Q0 + CH):
                        pvc = psC.tile([P, 2, CH], f32, tag="c")
                        pv = pvc[:, 0, :]
                        nmm = 0
                        for half in halves:
                            pt = ptp.tile([P, 8, CH], bf, tag="pt")
                            for tp in range(4):
                                tb = half * 8 + 2 * tp
                                sc = psA.tile([P, 2, CH], f32, tag="a")
                                for k2 in range(2):
                                    nc.tensor.matmul(
                                        sc[:, k2, :],
                                        qkvh[:, 1, (tb + k2) * P:(tb + k2 + 1) * P],
                                        qkvh[:, 0, cs:cs + CH],
                                        start=True, stop=True)
                                nc.scalar.activation(pt[:, 2 * tp:2 * tp + 2, :],
                                                     sc[:, 0:2, :], AF.Exp)
                                for k2 in range(2):
                                    nc.tensor.matmul(pv, vaug[:, tb + k2, :],
                                                     pt[:, 2 * tp + k2, :],
                                                     start=(nmm == 0),
                                                     stop=(nmm == T // P - 1))
                                    nmm += 1
                        # normalize by denominator (row 96 of pv)
                        dn = scr.tile([1, CH], bf, tag="scrb")
                        nc.vector.tensor_copy(dn[:], pv[DH:DH + 1, :])
                        nc.tensor.matmul(pvc[:, 1, :], ones_bf, dn[:],
                                         start=True, stop=True)
                        rc = scr.tile([P, CH], f32, tag="scr")
                        nc.vector.reciprocal_approx_fast(rc[:], pvc[:, 1, :])
                        nc.vector.tensor_mul(aout[:, h, cs - Q0:cs - Q0 + CH], pv, rc[:])

                # ---------- Wo + residual ----------
                for cs in (Q0, Q0 + CH):
                    for mp in range(KD // 2):
                        ps = psA.tile([P, 2, CH], f32, tag="a")
                        for bank in range(2):
                            m = 2 * mp + bank
                            for kb in range(NH):
                                nc.tensor.matmul(ps[:, bank, :],
                                                 wot[:, kb, m * P:(m + 1) * P],
                                                 aout[:, kb, cs - Q0:cs - Q0 + CH],
                                                 start=(kb == 0), stop=(kb == NH - 1))
                        for bank in range(2):
                            m = 2 * mp + bank
                            t = scr.tile([P, CH], f32, tag="scr")
                            nc.scalar.activation(t[:], ps[:, bank, :], AF.Identity,
                                                 bias=bo_t[:, m:m + 1])
                            nc.vector.tensor_add(x[:, m, cs:cs + CH], x[:, m, cs:cs + CH],
                                                 t[:])

                # ---------- LN2 + FFN + residual ----------
                layer_norm([Q0, Q0 + CH])
                for ci, cs in enumerate((Q0, Q0 + CH)):
                    ht = hp.tile([P, HB, CH], ffn_wdt, tag="h")
                    for hp2 in range(HB // 2):
                        w1t = w1p.tile([P, KD, 2 * P], ffn_wdt, tag="w1")
                        nc.sync.dma_start(
                            w1t[:], w1[l, :, hp2 * 2 * P:(hp2 + 1) * 2 * P].rearrange(
                                "(kt p) c -> p kt c", p=P))
                        ph = psA.tile([P, 2, CH], f32, tag="a")
                        for bank in range(2):
                            hb = 2 * hp2 + bank
                            if USE_FP8_FFN:
                                for j in range(KD // 2):
                                    nc.tensor.matmul(
                                        ph[:, bank, :],
                                        w1t[:, 2 * j:2 * j + 2, bank * P:(bank + 1) * P],
                                        xln[:, 2 * j:2 * j + 2, cs:cs + CH],
                                        start=(j == 0), stop=(j == KD // 2 - 1),
                                        perf_mode=mybir.MatmulPerfMode.DoubleRow)
                            else:
                                for kt in range(KD):
                                    nc.tensor.matmul(ph[:, bank, :],
                                                     w1t[:, kt, bank * P:(bank + 1) * P],
                                                     xln[:, kt, cs:cs + CH],
                                                     start=(kt == 0), stop=(kt == KD - 1))
                        for bank in range(2):
                            hb = 2 * hp2 + bank
                            nc.scalar.activation(ht[:, hb, :], ph[:, bank, :], AF.Gelu,
                                                 bias=b1_t[:, hb:hb + 1],
                                                 scale=s1_t[:, hb:hb + 1])
                    # W2: 6 output blocks in parallel chains (2 psA pairs + 2 psD)
                    pa0 = psA.tile([P, 2, CH], f32, tag="a")
                    pa1 = psA.tile([P, 2, CH], f32, tag="a")
                    pa2 = psC.tile([P, 2, CH], f32, tag="c")
                    chains = [pa0[:, 0, :], pa0[:, 1, :], pa1[:, 0, :], pa1[:, 1, :],
                              pa2[:, 0, :], pa2[:, 1, :]]
                    if USE_FP8_FFN:
                        for j in range(HB // 2):
                            w2t = w2p.tile([P, 2, D], ffn_wdt, tag="w2")
                            nc.sync.dma_start(
                                w2t[:], w2[l, 2 * j * P:(2 * j + 2) * P, :].rearrange(
                                    "(two p) d -> p two d", p=P))
                            for m in range(KD):
                                nc.tensor.matmul(
                                    chains[m], w2t[:, :, m * P:(m + 1) * P],
                                    ht[:, 2 * j:2 * j + 2, :],
                                    start=(j == 0), stop=(j == HB // 2 - 1),
                                    perf_mode=mybir.MatmulPerfMode.DoubleRow)
                    else:
                        for kb in range(HB):
                            w2t = w2p.tile([P, 2, D], ffn_wdt, tag="w2")
                            nc.sync.dma_start(
                                w2t[:, 0, :], w2[l, kb * P:(kb + 1) * P, :])
                            for m in range(KD):
                                nc.tensor.matmul(chains[m], w2t[:, 0, m * P:(m + 1) * P],
                                                 ht[:, kb, :],
                                                 start=(kb == 0), stop=(kb == HB - 1))
                    for m in range(KD):
                        t = scr.tile([P, CH], f32, tag="scr")
                        nc.vector.tensor_scalar(t[:], chains[m],
                                                s2_t[:, m:m + 1], b2_t[:, m:m + 1],
                                                ALU.mult, ALU.add)
                        nc.vector.tensor_add(x[:, m, cs:cs + CH],
                                             x[:, m, cs:cs + CH], t[:])

                    if l == 0:
                        # ship this chunk of x1 own-half to the peer, feature-major
                        cc_in = drp.tile([KD * P, CH], f16, tag=f"cci{ci}",
                                         name=f"cci{ci}")
                        cc_out = drp.tile([2 * KD * P, CH], f16, tag=f"cco{ci}",
                                          name=f"cco{ci}")
                        CC[ci] = cc_out
                        for kt in range(KD):
                            nc.sync.dma_start(cc_in[kt * P:(kt + 1) * P, :],
                                              x[:, kt, cs:cs + CH])
                        nc.gpsimd.collective_compute(
                            "AllGather",
                            mybir.AluOpType.bypass,
                            replica_groups=[[0, 1], [2, 3], [4, 5], [6, 7]],
                            ins=[cc_in.opt()],
                            outs=[cc_out.opt()],
                        )

            # ---------- final LN + decoder head ----------
            # final LN writes bf16 into the (now free) aout tile
            def to_aout(kt, cs, xc, pr_ap, eng):
                c0 = cs - Q0
                eng.tensor_mul(aout[:, kt, c0:c0 + CH], xc[:], pr_ap)

            layer_norm([Q0, Q0 + CH], to_aout)

            wdect = const.tile([P, KD, 6 * P], bf, tag="wdec")
            nc.sync.dma_start(wdect[:], wdec.rearrange("(kt p) c -> p kt c", p=P))
            bd_t = biasp.tile([P, KD], f32, tag="bd")
            nc.sync.dma_start(bd_t[:], bdec[:])
            for ci in range(2):
                yT = yp.tile([P, KD, CH], f16, tag="yT")
                for mp in range(KD // 2):
                    ps = psA.tile([P, 2, CH], f32, tag="a")
                    for bank in range(2):
                        m = 2 * mp + bank
                        for kt in range(KD):
                            nc.tensor.matmul(ps[:, bank, :],
                                             wdect[:, kt, m * P:(m + 1) * P],
                                             aout[:, kt, ci * CH:(ci + 1) * CH],
                                             start=(kt == 0), stop=(kt == KD - 1))
                    for bank in range(2):
                        m = 2 * mp + bank
                        nc.scalar.activation(yT[:, m, :], ps[:, bank, :], AF.Identity,
                                             bias=bd_t[:, m:m + 1])
                for kt in range(KD):
                    nc.sync.dma_start(y[kt, :, ci * CH:(ci + 1) * CH], yT[:, kt, :])

    nc.compile()
    return nc


def _quant_rows(w):
    """Per-output-row e4m3 quantization. w: [out, in] -> (wq fp8, scale[out])"""
    amax = np.maximum(np.abs(w).max(axis=1), 1e-20)
    s = 192.0 / amax
    wq = (w * s[:, None]).astype(F8)
    return wq, (1.0 / s).astype(np.float32)


def _prep_weights(inputs):
    """Host-side weight folding/packing. Returns dict of shared arrays."""
    g1, be1 = inputs["gamma1"], inputs["beta1"]
    g2, be2 = inputs["gamma2"], inputs["beta2"]
    Wqkv, bqkv = inputs["Wqkv"], inputs["bqkv"]
    Wo, bo = inputs["Wo"], inputs["bo"]
    W1, b1 = inputs["W1"], inputs["b1"]
    W2, b2 = inputs["W2"], inputs["b2"]
    gn, gb = inputs["gn"], inputs["gb"]
    Wdec, bdec = inputs["Wdec"], inputs["bdec"]

    qdt = F8 if USE_FP8_QKV else BF16
    fdt = F8 if USE_FP8_FFN else BF16
    wqkv_a = np.zeros((DEPTH, D, NH * 3 * P), qdt)
    bqkv_a = np.zeros((DEPTH, NH * 3, P), np.float32)
    sqkv_a = np.ones((DEPTH, NH * 3, P), np.float32)
    wo_a = np.zeros((DEPTH, NH * P, D), np.float32)
    bwo_a = np.zeros((DEPTH, KD, P), np.float32)
    w1_a = np.zeros((DEPTH, D, HID), fdt)
    b1_a = np.zeros((DEPTH, HB, P), np.float32)
    s1_a = np.ones((DEPTH, HB, P), np.float32)
    w2_a = np.zeros((DEPTH, HID, D), fdt)
    b2_a = np.zeros((DEPTH, KD, P), np.float32)
    s2_a = np.ones((DEPTH, KD, P), np.float32)
    scale = 1.0 / np.sqrt(DH)
    for l in range(DEPTH):
        Wp = Wqkv[l] * g1[l][None, :]                  # fold gamma1
        bp = bqkv[l] + Wqkv[l] @ be1[l]                # fold beta1
        Wp = Wp.copy()
        bp = bp.copy()
        Wp[:D] *= scale                                # fold 1/sqrt(dh) into Q
        bp[:D] *= scale
        if USE_FP8_QKV:
            Wpq, Wps = _quant_rows(Wp)
        else:
            Wpq, Wps = Wp.astype(BF16), np.ones(3 * D, np.float32)
        for h in range(NH):
            for c in range(3):                         # q,k,v
                rows = slice(c * D + h * DH, c * D + (h + 1) * DH)
                wqkv_a[l, :, (h * 3 + c) * P:(h * 3 + c) * P + DH] = Wpq[rows].T
                bqkv_a[l, h * 3 + c, :DH] = bp[rows]
                sqkv_a[l, h * 3 + c, :DH] = Wps[rows]
            bqkv_a[l, h * 3 + 2, DH] = 1.0             # ones-row -> denominators
            wo_a[l, h * P:h * P + DH, :] = Wo[l][:, h * DH:(h + 1) * DH].T
        bwo_a[l] = bo[l].reshape(KD, P)
        W1f = W1[l] * g2[l][None, :]
        b1f = b1[l] + W1[l] @ be2[l]
        if USE_FP8_FFN:
            W1q, W1s = _quant_rows(W1f)
            W2q, W2s = _quant_rows(W2[l])
        else:
            W1q, W1s = W1f.astype(BF16), np.ones(HID, np.float32)
            W2q, W2s = W2[l].astype(BF16), np.ones(D, np.float32)
        w1_a[l] = W1q.T
        b1_a[l] = b1f.reshape(HB, P)
        s1_a[l] = W1s.reshape(HB, P)
        w2_a[l] = W2q.T
        b2_a[l] = b2[l].reshape(KD, P)
        s2_a[l] = W2s.reshape(KD, P)
    wdec_a = (Wdec * gn[None, :]).T
    bdec_a = (bdec + Wdec @ gb).reshape(KD, P)
    tp = lambda a: np.ascontiguousarray(a.transpose(0, 2, 1))
    return {
        "wqkv": wqkv_a, "bqkv": tp(bqkv_a), "sqkv": tp(sqkv_a),
        "wo": wo_a.astype(BF16), "bwo": tp(bwo_a),
        "w1": w1_a, "b1": tp(b1_a), "s1": tp(s1_a),
        "w2": w2_a, "b2": tp(b2_a), "s2": tp(s2_a),
        "wdec": wdec_a.astype(BF16), "bdec": np.ascontiguousarray(bdec_a.T),
    }


def kernel(**inputs):
    from concourse.bass_utils import run_bass_kernel_spmd

    inputs = {k: np.asarray(v) for k, v in inputs.items()}
    if "nc" not in _cache:
        _cache["nc"] = _build()
    nc = _cache["nc"]

    shared = _prep_weights(inputs)
    mask = inputs["mask"]
    vt = inputs["visible_tokens"].astype(np.float32)
    mt = inputs["mask_token"].astype(np.float32)

    # host-side scatter: x0[b, t] = vt[b, idx] if mask else mask_token
    nv = np.clip(np.cumsum(mask.astype(np.int64), axis=1) - 1, 0, N_VIS - 1)
    gathered = np.take_along_axis(vt, nv[..., None], axis=1)
    x0_full = np.where(mask[..., None], gathered, mt[None, None, :])  # (B,T,D)

    in_maps = []
    for core in range(8):
        b, s = core // 2, core % 2
        if s == 0:
            perm = np.concatenate([np.arange(TQ, T), np.arange(0, TQ)])
        else:
            perm = np.arange(T)
        x0p = np.ascontiguousarray(
            x0_full[b][perm].T.astype(np.float16).reshape(KD, P, T))
        peer = 1 - s
        m = dict(shared)
        m["x0"] = x0p
        m["pidx"] = np.ascontiguousarray(
            (peer * D + np.arange(D, dtype=np.int32)).reshape(KD, P).T)
        in_maps.append(m)

    res = run_bass_kernel_spmd(nc, in_maps, core_ids=list(range(8)),
                               **_cache.get("run_kwargs", {}))
    _cache["last_results"] = res

    out = np.zeros((B, T, D), np.float32)
    for core in range(8):
        b, s = core // 2, core % 2
        yv = res.results[core]["y"].reshape(D, TQ).astype(np.float32)
        out[b, s * TQ:(s + 1) * TQ] = yv.T
    return out


if __name__ == "__main__":
    rng = np.random.default_rng(0)
    print("building...")
    _build()
    print("built ok")
